# revision 1
# baseline (speedup 1.0000x reference)
"""Distributed Trainium2 Bass kernel for AlignmentContrastiveLoss.

Reference computation (B=256, L_im=37, L_s=33, D=1024):
    im  = l2norm(im_set)[:, 1:, :]   masked by im_len-1     [B, 36, D]
    s   = l2norm(s_seq)[:, 1:-2, :]  masked by s_len-3      [B, 30, D]
    align[b,c,i,j] = im[b,i] . s[c,j]   (masked entries -> 0)
    scores[b,c] = sum_j max_i align[b,c,i,j]
    loss = sum_b relu(M + max_{c!=b} scores[b,c] - scores[b,b])
         + sum_c relu(M + max_{b!=c} scores[b,c] - scores[c,c])

Sharding: image batch axis across 8 cores (32 images/core); every core
holds the full sentence set (replicated via its input map).  Each core
computes its 32x256 block of scores via fp32r matmuls (PE), max-over-i
on DVE directly from PSUM, the j-sum via small 0/1 "G" matmuls into two
per-core scoresT accumulators [256 x 32], then per-core partial stats
(col-max / diag / row-hinge) are AllGathered (768 floats) and every core
redundantly computes the final scalar.  s norms are computed sharded and
AllGathered (960 floats each) instead of redundantly per-core.
"""

import os
import sys

import numpy as np

for _p in ("/opt/trn_rl_repo", "/root/.axon_site/_ro/trn_rl_repo"):
    if os.path.isdir(_p) and _p not in sys.path:
        sys.path.append(_p)

import concourse.bass as bass
import concourse.mybir as mybir
import concourse.tile as tile
from concourse import bacc
from concourse.bass_utils import run_bass_kernel_spmd


def _ensure_axon_hooks():
    """Some agent images ship an ``antenv`` without ``axon_hooks``, but
    bass_utils hard-imports it when trace=True.  Provide the registry and,
    when libaxon_pjrt.so is available, the real NTFF profile hook."""
    import types

    try:
        import antenv.axon_hooks  # noqa: F401
        return
    except ImportError:
        pass
    try:
        import antenv
    except ImportError:
        return
    mod = types.ModuleType("antenv.axon_hooks")
    mod._hook = None
    mod.set_axon_ntff_profile_hook = lambda h: setattr(mod, "_hook", h)
    mod.get_axon_ntff_profile_hook = lambda: mod._hook
    sys.modules["antenv.axon_hooks"] = mod
    antenv.axon_hooks = mod
    so_path = "/opt/axon/libaxon_pjrt.so"
    try:
        import trn_agent_boot.trn_boot as _tb
        if os.path.exists(so_path):
            mod._hook = _tb._ntff_profile_via_ctypes(so_path)
    except Exception:
        pass


_ensure_axon_hooks()

F32 = mybir.dt.float32
F32R = mybir.dt.float32r
BF16 = mybir.dt.bfloat16
I32 = mybir.dt.int32
AX = mybir.AxisListType
ALU = mybir.AluOpType
ACT = mybir.ActivationFunctionType

NCORES = 8
B, LI, LS, D = 256, 36, 30, 1024
BL = B // NCORES            # 32 images / core
BI = BL * LI                # 1152 im rows / core
CJ = B * LS                 # 7680 (c,j) rows
NT = CJ // 128              # 60 M-tiles
NRT = BI // 128             # 9 im row-tiles
KC = D // 128               # 8 contraction chunks
SJ = CJ // NCORES           # 960 s rows / core (norm shard)
WROWS = 960                 # rows per 32-sentence window
NCHUNKS = [(0, 432, 12), (432, 432, 12), (864, 288, 8)]  # (off, width, n_images)
MARGIN, EPS, NEG = 0.2, 1e-12, -1.0e9

LAST_RESULT = None  # BassKernelResults of the most recent run (for test harness)


# ---------------------------------------------------------------------------
# compile-time tables
# ---------------------------------------------------------------------------

HALF_T = NT // 2  # 30 M-tiles per 128-sentence half


def _gmat_host():
    """G[p, 128t + cl] = 1 where row (128t+p) belongs to local sentence cl
    of tile t's half; G_t.T @ mx_t sums words j into scoresT[half] rows."""
    g = np.zeros((128, NT * 128), np.float32)
    for t in range(NT):
        h = t // HALF_T
        p = np.arange(128)
        cl = (128 * t + p) // LS - 128 * h
        g[p, 128 * t + cl] = 1.0
    return g


def _core_masks(m):
    pos0 = np.zeros((128, 32), np.float32)
    pos1 = np.zeros((128, 32), np.float32)
    tgt = pos0 if m < 4 else pos1
    b = np.arange(32)
    tgt[32 * (m % 4) + b, b] = 1.0
    return pos0, pos1, np.ascontiguousarray(pos0.T), np.ascontiguousarray(pos1.T)


# ---------------------------------------------------------------------------
# device program
# ---------------------------------------------------------------------------

def build_nc():
    nc = bacc.Bacc(None, target_bir_lowering=False, debug=False, num_devices=NCORES)

    imr_e = nc.declare_dram_parameter("imr", [BI, D], F32, isOutput=False)
    snr_e = nc.declare_dram_parameter("snr", [SJ, D], F32, isOutput=False)
    st_e = nc.declare_dram_parameter("st", [NT, 128, KC, 128], F32, isOutput=False)
    imlen_e = nc.declare_dram_parameter("imlen", [BL], I32, isOutput=False)
    slen_e = nc.declare_dram_parameter("slen", [B], I32, isOutput=False)
    iota36_e = nc.declare_dram_parameter("iota36", [BL, LI], F32, isOutput=False)
    iota30_e = nc.declare_dram_parameter("iota30", [128, LS], F32, isOutput=False)
    ident_e = nc.declare_dram_parameter("ident", [128, 128], F32, isOutput=False)
    gmat_e = nc.declare_dram_parameter("gmat", [128, NT * 128], F32R, isOutput=False)
    pos0_e = nc.declare_dram_parameter("pos0", [128, 32], F32, isOutput=False)
    pos1_e = nc.declare_dram_parameter("pos1", [128, 32], F32, isOutput=False)
    post0_e = nc.declare_dram_parameter("post0", [32, 128], F32, isOutput=False)
    post1_e = nc.declare_dram_parameter("post1", [32, 128], F32, isOutput=False)
    out_e = nc.declare_dram_parameter("out", [1, 1], F32, isOutput=True)

    with tile.TileContext(nc) as tc:
        from contextlib import ExitStack

        with ExitStack() as ctx:
            dram = ctx.enter_context(tc.tile_pool(name="dram", bufs=1, space="DRAM"))
            const = ctx.enter_context(tc.tile_pool(name="const", bufs=1))
            small = ctx.enter_context(tc.tile_pool(name="small", bufs=1))
            stp = ctx.enter_context(tc.tile_pool(name="stp", bufs=3))
            mxp = ctx.enter_context(tc.tile_pool(name="mxp", bufs=4))
            prep = ctx.enter_context(tc.tile_pool(name="prep", bufs=3))
            # PSUM budget (8 banks): align 6 + S accumulator 1 + epi scratch 1
            pal = ctx.enter_context(tc.tile_pool(name="pal", bufs=5, space="PSUM"))

            # DRAM scratch
            imask_d = dram.tile([BI, 1], F32, tag="imask_d")
            smask_d = dram.tile([CJ, 1], F32, tag="smask_d")
            snorm_d = dram.tile([SJ, 1], F32, tag="snorm_d")
            snormall_d = dram.tile([CJ, 1], F32, tag="snormall_d")
            pay_d = dram.tile([128, 6], F32, tag="pay_d")
            ag2_d = dram.tile([NCORES * 128, 6], F32, tag="ag2_d")

            def epi_psum(shape, name):
                return pal.tile(shape, F32, tag="epi", bufs=1, name=name)

            # ---- early consts needed by prep ----
            ident = const.tile([128, 128], F32, tag="ident")
            nc.sync.dma_start(out=ident[:, :], in_=ident_e[:, :])
            iota36 = const.tile([BL, LI], F32, tag="iota36")
            nc.sync.dma_start(out=iota36[:, :], in_=iota36_e[:, :])
            iota30 = const.tile([128, LS], F32, tag="iota30")
            nc.sync.dma_start(out=iota30[:, :], in_=iota30_e[:, :])

            # ---- masks from lengths ----
            imlen_i = small.tile([BL, 1], I32, tag="imlen_i")
            nc.sync.dma_start(out=imlen_i[:, :], in_=imlen_e[:])
            imlen_f = small.tile([BL, 1], F32, tag="imlen_f")
            nc.vector.tensor_copy(imlen_f[:, :], imlen_i[:, :])
            nc.vector.tensor_scalar_add(imlen_f[:, :], imlen_f[:, :], -1.0)
            mask36 = small.tile([BL, LI], F32, tag="mask36")
            nc.vector.tensor_scalar(
                out=mask36[:, :], in0=iota36[:, :], scalar1=imlen_f[:, :],
                scalar2=None, op0=ALU.is_lt,
            )
            nc.sync.dma_start(
                out=imask_d.rearrange("(b i) o -> b (i o)", b=BL),
                in_=mask36[:, :],
            )
            # imask reload via [9,128] contiguous load + PE transpose
            imask9 = small.tile([NRT, 128], F32, tag="imask9")
            nc.sync.dma_start(
                out=imask9[:, :],
                in_=imask_d.rearrange("(rt p) o -> rt (p o)", rt=NRT),
            )
            imaskT_ps = epi_psum([128, NRT], "imaskT_ps")
            nc.tensor.transpose(imaskT_ps[:, :], imask9[:, :], ident[0:NRT, 0:NRT])
            imask_sb = small.tile([128, NRT], F32, tag="imask_sb")
            nc.scalar.copy(imask_sb[:, :], imaskT_ps[:, :])


            slen_i = small.tile([128, 2], I32, tag="slen_i")
            nc.sync.dma_start(
                out=slen_i[:, :],
                in_=slen_e.ap().rearrange("(h c) -> c h", h=2),
            )
            slen_f = small.tile([128, 2], F32, tag="slen_f")
            nc.vector.tensor_copy(slen_f[:, :], slen_i[:, :])
            nc.vector.tensor_scalar_add(slen_f[:, :], slen_f[:, :], -3.0)
            for h in range(2):
                mask30 = small.tile([128, LS], F32, tag="mask30")
                nc.vector.tensor_scalar(
                    out=mask30[:, :], in0=iota30[:, :], scalar1=slen_f[:, h:h + 1],
                    scalar2=None, op0=ALU.is_lt,
                )
                nc.sync.dma_start(
                    out=smask_d[3840 * h:3840 * (h + 1), :]
                    .rearrange("(c j) o -> c (j o)", c=128),
                    in_=mask30[:, :],
                )

            # ---- phase 1+2 interleaved: im norms first (critical path), s norms after ----
            imr_tiles = []
            imssqs = []
            for rt in range(NRT):
                imr_t = prep.tile([128, D], F32, tag="imld", name="imr_t", bufs=9)
                nc.sync.dma_start(out=imr_t[:, :], in_=imr_e[128 * rt:128 * (rt + 1), :])
                sq = prep.tile([128, D], F32, tag="imsq", name="sq")
                ssq = small.tile([128, 1], F32, tag=f"imssq{rt}", name="ssq")
                nc.scalar.activation(sq[:, :], imr_t[:, :], ACT.Square,
                                     accum_out=ssq[:, :])
                imr_tiles.append(imr_t)
                imssqs.append(ssq)

            # sharded s sum-of-squares + AllGather (GpSimd squares, DVE reduces)
            ssq8 = small.tile([120, 8], F32, tag="ssq8")
            for j in range(8):
                snr_t = prep.tile([120, D], F32, tag="sld", name="snr_t")
                nc.sync.dma_start(out=snr_t[:, :], in_=snr_e[120 * j:120 * (j + 1), :])
                sq = prep.tile([120, D], F32, tag="ssq", name="sq")
                nc.gpsimd.tensor_mul(sq[:, :], snr_t[:, :], snr_t[:, :])
                nc.vector.tensor_reduce(out=ssq8[:, j:j + 1], in_=sq[:, :],
                                        axis=AX.X, op=ALU.add)
            # [120, 8] -> [8, 120] so the DRAM write is contiguous per partition
            ssqT_ps = epi_psum([8, 120], "ssqT_ps")
            nc.tensor.transpose(ssqT_ps[:, :], ssq8[:, :], ident[0:120, 0:120])
            ssqT = small.tile([8, 120], F32, tag="ssqT")
            nc.scalar.copy(ssqT[:, :], ssqT_ps[:, :])
            nc.sync.dma_start(
                out=snorm_d.rearrange("(j p) o -> j (p o)", j=8),
                in_=ssqT[:, :],
            )
            nc.gpsimd.collective_compute(
                "AllGather", ALU.bypass,
                replica_groups=[list(range(NCORES))],
                ins=[snorm_d.opt()],
                outs=[snormall_d.opt()],
            )

            # bf16 identity for fast prep transposes
            ident_bf = const.tile([128, 128], BF16, tag="ident_bf")
            nc.scalar.copy(ident_bf[:, :], ident[:, :])

            # finish im prep: scale (DVE), cast to bf16 (ACT), transpose (PE, bf16)
            imt = const.tile([128, KC * BI], BF16, tag="imt")
            for rt in range(NRT):
                imr_t = imr_tiles[rt]
                ssq = imssqs[rt]
                nrm = small.tile([128, 1], F32, tag="imnrm")
                nc.scalar.activation(nrm[:, :], ssq[:, :], ACT.Sqrt)
                nc.vector.tensor_scalar_max(nrm[:, :], nrm[:, :], EPS)
                rcp = small.tile([128, 1], F32, tag="imrcp")
                nc.vector.reciprocal(rcp[:, :], nrm[:, :])
                ims_bf = prep.tile([128, D], BF16, tag="imsbf", name="ims_bf")
                nc.vector.tensor_scalar(
                    out=ims_bf[:, :], in0=imr_t[:, :], scalar1=rcp[:, :],
                    scalar2=imask_sb[:, rt:rt + 1], op0=ALU.mult, op1=ALU.mult,
                )
                for k in range(KC):
                    pst = pal.tile([128, 128], BF16, tag="al", name="pst")
                    nc.tensor.transpose(pst[:, :], ims_bf[:, 128 * k:128 * (k + 1)],
                                        ident_bf[:, :])
                    dst = imt[:, BI * k + 128 * rt:BI * k + 128 * (rt + 1)]
                    if k % 2 == 0:
                        nc.vector.tensor_copy(dst, pst[:, :])
                    else:
                        nc.scalar.copy(dst, pst[:, :])

            # ---- late consts (needed by main loop G-matmuls / epilogue) ----
            gmat = const.tile([128, NT * 128], F32R, tag="gmat")
            nc.sync.dma_start(out=gmat[:, :], in_=gmat_e[:, :])
            pos0 = const.tile([128, 32], F32, tag="pos0")
            nc.sync.dma_start(out=pos0[:, :], in_=pos0_e[:, :])
            pos1 = const.tile([128, 32], F32, tag="pos1")
            nc.sync.dma_start(out=pos1[:, :], in_=pos1_e[:, :])
            post0 = const.tile([32, 128], F32, tag="post0")
            nc.sync.dma_start(out=post0[:, :], in_=post0_e[:, :])
            post1 = const.tile([32, 128], F32, tag="post1")
            nc.sync.dma_start(out=post1[:, :], in_=post1_e[:, :])
            ones128 = const.tile([128, 1], F32, tag="ones128")
            nc.gpsimd.memset(ones128[:, :], 1.0)
            margin128 = const.tile([128, 1], F32, tag="margin128")
            nc.gpsimd.memset(margin128[:, :], MARGIN)

            # ---- phase 4: main loop over 60 M-tiles ----
            # S halves share one PSUM bank: [128, 64], cols [0:32] half0, [32:64] half1
            psacc = ctx.enter_context(tc.tile_pool(name="psacc", bufs=1, space="PSUM"))
            s_ps = [psacc.tile([128, 32], F32, tag=f"S{h}", name=f"S{h}")[:, :]
                    for h in range(2)]

            # epilogue constants + buffers (ready before the loop so half-0
            # stats can run as soon as S0 completes at t=30)
            posm = [pos0, pos1]
            payload = small.tile([128, 6], F32, tag="payload")
            snd = [small.tile([128, 32], F32, tag=f"snd{h}", name=f"snd{h}")
                   for h in range(2)]
            trash = small.tile([128, 32], F32, tag="trash")
            negm = [small.tile([128, 32], F32, tag=f"negm{h}", name=f"negm{h}")
                    for h in range(2)]
            nc.vector.tensor_scalar_mul(negm[0][:, :], pos0[:, :], NEG)
            nc.vector.tensor_scalar_mul(negm[1][:, :], pos1[:, :], NEG)
            posr = [small.tile([128, 32], F32R, tag=f"posr{h}", name=f"posr{h}")
                    for h in range(2)]
            nc.scalar.copy(posr[0][:, :], pos0[:, :])
            nc.scalar.copy(posr[1][:, :], pos1[:, :])
            postr = [small.tile([32, 128], F32R, tag=f"postr{h}", name=f"postr{h}")
                     for h in range(2)]
            nc.scalar.copy(postr[0][:, :], post0[:, :])
            nc.scalar.copy(postr[1][:, :], post1[:, :])
            onesr = const.tile([128, 1], F32R, tag="onesr")
            nc.scalar.copy(onesr[:, :], ones128[:, :])
            rm = small.tile([32, 2], F32, tag="rm")

            def emit_stats_h(h):
                # diag extraction: accum_out = sum(S * pos) -> payload col 2+h
                nc.vector.scalar_tensor_tensor(
                    out=trash[:, :], in0=s_ps[h], scalar=1.0, in1=posm[h][:, :],
                    op0=ALU.mult, op1=ALU.mult, accum_out=payload[:, 2 + h:3 + h],
                )
                nc.vector.tensor_add(snd[h][:, :], s_ps[h], negm[h][:, :])
                nc.vector.tensor_reduce(out=payload[:, h:h + 1], in_=snd[h][:, :],
                                        axis=AX.X, op=ALU.max)
                stp_ps = epi_psum([32, 128], "stp_ps")
                nc.tensor.transpose(stp_ps[:, :], snd[h][:, :], ident[:, :])
                nc.vector.tensor_reduce(out=rm[:, h:h + 1], in_=stp_ps[:, :],
                                        axis=AX.X, op=ALU.max)

            def emit_scale_g(t):
                mx, _ = pending[t]
                mx_r = mxp.tile([128, 32], F32R, tag="mx_r", name="mx_r")
                nc.scalar.mul(mx_r[:, :], mx[:, :], mul=sscale[:, t:t + 1])
                nc.tensor.matmul(
                    s_ps[t // HALF_T],
                    lhsT=gmat[:, 128 * t:128 * (t + 1)],
                    rhs=mx_r[:, :],
                    start=(t % HALF_T == 0), stop=(t % HALF_T == HALF_T - 1),
                )

            pending = {}
            next_g = [0]

            def drain_g(upto):
                while next_g[0] <= upto:
                    emit_scale_g(next_g[0])
                    next_g[0] += 1

            for t in range(NT):
                st_t = stp.tile([128, KC * 128], F32, tag="st")
                nc.sync.dma_start(
                    out=st_t.rearrange("p (k c) -> p k c", k=KC),
                    in_=st_e[t, :, :, :],
                )
                st_bf = stp.tile([128, KC * 128], BF16, tag="st_bf")
                nc.scalar.copy(st_bf[:, :], st_t[:, :])
                # k-outer: one weight per (t,k) feeds all 3 N-chunks
                ps3 = [pal.tile([128, 432], F32, tag="al", name="ps") for _ in range(3)]
                for k in range(KC):
                    for ci, (noff, nw, nimg) in enumerate(NCHUNKS):
                        nc.tensor.matmul(
                            ps3[ci][:, :nw],
                            lhsT=st_bf[:, 128 * k:128 * (k + 1)],
                            rhs=imt[:, BI * k + noff:BI * k + noff + nw],
                            start=(k == 0), stop=(k == KC - 1),
                        )
                mx = mxp.tile([128, 32], F32, tag="mx", name="mx")
                ioff = 0
                for ci, (noff, nw, nimg) in enumerate(NCHUNKS):
                    nc.vector.tensor_reduce(
                        out=mx[:, ioff:ioff + nimg],
                        in_=ps3[ci].rearrange("p (g i) -> p g i", i=LI)[:, :nimg, :],
                        axis=AX.X, op=ALU.max,
                    )
                    ioff += nimg
                pending[t] = (mx, None)
                if t == 12:
                    # ---- phase 3: s scale vector [128, NT] ----
                    # contiguous [60,128] loads + PE transposes (avoid 4B-strided DMA)
                    ssq60 = small.tile([NT, 128], F32, tag="ssq60")
                    nc.sync.dma_start(
                        out=ssq60[:, :],
                        in_=snormall_d.rearrange("(t p) o -> t (p o)", t=NT),
                    )
                    ssqall_ps = epi_psum([128, NT], "ssqall_ps")
                    nc.tensor.transpose(ssqall_ps[:, :], ssq60[:, :], ident[0:NT, 0:NT])
                    ssqall = small.tile([128, NT], F32, tag="ssqall")
                    nc.scalar.copy(ssqall[:, :], ssqall_ps[:, :])

                    smask60 = small.tile([NT, 128], F32, tag="smask60")
                    nc.sync.dma_start(
                        out=smask60[:, :],
                        in_=smask_d.rearrange("(t p) o -> t (p o)", t=NT),
                    )
                    smask_ps = epi_psum([128, NT], "smask_ps")
                    nc.tensor.transpose(smask_ps[:, :], smask60[:, :], ident[0:NT, 0:NT])
                    smask_sb = small.tile([128, NT], F32, tag="smask_sb")
                    nc.scalar.copy(smask_sb[:, :], smask_ps[:, :])

                    snrm = small.tile([128, NT], F32, tag="snrm")
                    nc.scalar.activation(snrm[:, :], ssqall[:, :], ACT.Sqrt)
                    nc.vector.tensor_scalar_max(snrm[:, :], snrm[:, :], EPS)
                    sscale = small.tile([128, NT], F32, tag="sscale")
                    nc.vector.reciprocal(sscale[:, :], snrm[:, :])
                    nc.vector.tensor_mul(sscale[:, :], sscale[:, :], smask_sb[:, :])


                # defer scale+G; sscale (AllGather #1) is only ready ~t=13
                if t >= 13:
                    drain_g(t - 2)
                if t == HALF_T + 3:
                    emit_stats_h(0)
            drain_g(NT - 1)

            # ---- phase 5: half-1 stats + AllGather + final ----
            emit_stats_h(1)
            dcolr = small.tile([128, 2], F32R, tag="dcolr")
            nc.scalar.copy(dcolr[:, :], payload[:, 2:4])
            rowmax = small.tile([32, 1], F32, tag="rowmax")
            nc.vector.tensor_max(rowmax[:, :], rm[:, 0:1], rm[:, 1:2])
            # diag in row order; N=2 (fp32r needs even moving dim), cross terms 0
            dfree_ps = epi_psum([32, 2], "dfree_ps")
            nc.tensor.matmul(dfree_ps[:, :], lhsT=posr[0][:, :],
                             rhs=dcolr[:, :], start=True, stop=False)
            nc.tensor.matmul(dfree_ps[:, :], lhsT=posr[1][:, :],
                             rhs=dcolr[:, :], start=False, stop=True)
            dfree2 = small.tile([32, 2], F32, tag="dfree2")
            nc.scalar.copy(dfree2[:, :], dfree_ps[:, :])
            dfree_sb = small.tile([32, 1], F32, tag="dfree_sb")
            nc.vector.tensor_add(dfree_sb[:, :], dfree2[:, 0:1], dfree2[:, 1:2])
            rh_pre = small.tile([32, 2], F32, tag="rh_pre")
            nc.gpsimd.memset(rh_pre[:, :], 0.0)
            nc.vector.tensor_sub(rh_pre[:, 0:1], rowmax[:, :], dfree_sb[:, :])
            rowhinge = small.tile([32, 2], F32R, tag="rowhinge")
            nc.scalar.activation(rowhinge[:, :], rh_pre[:, :], ACT.Relu,
                                 bias=margin128[0:32, :])
            for h in range(2):
                rh_ps = epi_psum([128, 2], "rh_ps")
                nc.tensor.matmul(rh_ps[:, :], lhsT=postr[h][:, :],
                                 rhs=rowhinge[:, :], start=True, stop=True)
                nc.scalar.copy(payload[:, 4 + h:5 + h], rh_ps[:, 0:1])

            # payload -> DRAM (one DMA, contiguous per partition) -> AllGather
            nc.sync.dma_start(out=pay_d[:, :], in_=payload[:, :])
            nc.gpsimd.collective_compute(
                "AllGather", ALU.bypass,
                replica_groups=[list(range(NCORES))],
                ins=[pay_d.opt()],
                outs=[ag2_d.opt()],
            )

            # final combine (identical on every core)
            ag_sb = small.tile([NCORES, 768], F32, tag="ag_sb")
            nc.sync.dma_start(
                out=ag_sb[:, :],
                in_=ag2_d.rearrange("(m p) c -> m (p c)", m=NCORES),
            )
            agv = ag_sb.rearrange("m (p c) -> m p c", c=6)
            finalvec = small.tile([128, 4], F32R, tag="finalvec")
            agg = small.tile([128, 6], F32, tag="agg")
            for c6 in range(6):
                agt = small.tile([NCORES, 128], F32, tag="agt")
                nc.vector.tensor_copy(agt[:, :], agv[:, :, c6])
                t_ps = epi_psum([128, NCORES], "t_ps")
                nc.tensor.transpose(t_ps[:, :], agt[:, :],
                                    ident[0:NCORES, 0:NCORES])
                nc.vector.tensor_reduce(
                    out=agg[:, c6:c6 + 1], in_=t_ps[:, :], axis=AX.X,
                    op=(ALU.max if c6 < 2 else ALU.add),
                )
            for h in range(2):
                # colhinge_h = relu(colmax_h - dfull_h + margin)
                ch = small.tile([128, 1], F32, tag="ch")
                nc.vector.tensor_sub(ch[:, :], agg[:, h:h + 1], agg[:, 2 + h:3 + h])
                nc.scalar.activation(finalvec[:, h:h + 1], ch[:, :], ACT.Relu,
                                     bias=margin128[:, :])
                nc.scalar.copy(finalvec[:, 2 + h:3 + h], agg[:, 4 + h:5 + h])
            fin_ps = epi_psum([1, 4], "fin_ps")
            nc.tensor.matmul(fin_ps[:, :], lhsT=onesr[:, :],
                             rhs=finalvec[:, :], start=True, stop=True)
            loss = small.tile([1, 1], F32, tag="loss")
            nc.vector.tensor_reduce(out=loss[:, :], in_=fin_ps[:, :], axis=AX.X,
                                    op=ALU.add)
            nc.sync.dma_start(out=out_e[:, :], in_=loss[:, :])

    nc.finalize()
    return nc


# ---------------------------------------------------------------------------
# host side
# ---------------------------------------------------------------------------

def build_in_maps(im_set, s_seq, im_len, s_len):
    im_set = np.asarray(im_set, dtype=np.float32)
    s_seq = np.asarray(s_seq, dtype=np.float32)
    im_len = np.asarray(im_len, dtype=np.int32)
    s_len = np.asarray(s_len, dtype=np.int32)

    s_rows = np.ascontiguousarray(s_seq[:, 1:1 + LS, :].reshape(CJ, D))
    # st[t, p, k, c] = s_rows[128t + c, 128k + p]
    st = np.ascontiguousarray(
        s_rows.reshape(NT, 128, KC, 128).transpose(0, 3, 2, 1))
    gmat = _gmat_host()
    iota36 = np.broadcast_to(np.arange(LI, dtype=np.float32), (BL, LI)).copy()
    iota30 = np.broadcast_to(np.arange(LS, dtype=np.float32), (128, LS)).copy()
    ident = np.eye(128, dtype=np.float32)

    in_maps = []
    for m in range(NCORES):
        pos0, pos1, post0, post1 = _core_masks(m)
        imr = np.ascontiguousarray(
            im_set[BL * m:BL * (m + 1), 1:, :].reshape(BI, D))
        snr = np.ascontiguousarray(s_rows[SJ * m:SJ * (m + 1)])
        in_maps.append({
            "imr": imr,
            "snr": snr,
            "st": st,
            "imlen": np.ascontiguousarray(im_len[BL * m:BL * (m + 1)]),
            "slen": s_len,
            "iota36": iota36,
            "iota30": iota30,
            "ident": ident,
            "gmat": gmat,
            "pos0": pos0,
            "pos1": pos1,
            "post0": post0,
            "post1": post1,
        })
    return in_maps


_NC_CACHE = None


def kernel(im_set, s_seq, im_len, s_len):
    global _NC_CACHE, LAST_RESULT
    if _NC_CACHE is None:
        _NC_CACHE = build_nc()
    nc = _NC_CACHE
    in_maps = build_in_maps(im_set, s_seq, im_len, s_len)
    res = run_bass_kernel_spmd(nc, in_maps, core_ids=list(range(NCORES)))
    LAST_RESULT = res
    out = np.asarray(res.results[0]["out"], dtype=np.float32).reshape(())
    return out



# revision 13
# speedup vs baseline: 2.0011x; 2.0011x over previous
"""Distributed Trainium2 Bass kernel for AlignmentContrastiveLoss (packed).

Reference computation (B=256, L_im=37, L_s=33, D=1024):
    im  = l2norm(im_set)[:, 1:, :]   masked by im_len-1     [B, 36, D]
    s   = l2norm(s_seq)[:, 1:-2, :]  masked by s_len-3      [B, 30, D]
    align[b,c,i,j] = im[b,i] . s[c,j]   (masked entries -> 0)
    scores[b,c] = sum_j max_i align[b,c,i,j]
    loss = sum_b relu(M + max_{c!=b} scores[b,c] - scores[b,b])
         + sum_c relu(M + max_{b!=c} scores[b,c] - scores[c,c])

Sparsity exploitation (the big win vs a dense kernel): only valid im
regions / s words are ever loaded or multiplied.
  * s side: all valid (c,j) rows are packed densely (per 128-sentence
    half, zero-padded to 128-row tiles) -> NT ~ 36 instead of 60 tiles.
    Invalid words contribute exactly 0 to scores, so dropping them is
    exact; the host-built 0/1 G matrix maps packed rows -> sentences.
  * im side: images are sorted by region count and dealt round-robin
    (rank r -> core r%8, slot r//8) so all 8 cores share one compiled
    slot profile; slot lengths are the per-group max quantized to
    multiples of 4 (<= 7 distinct lengths -> few DVE reduce runs).
    Images with im_l < 36 get >= 1 zero pad row in their slot, which
    reproduces the reference's max-with-0 semantics exactly.
  * per-core matmul: s packed rows stationary (bf16, host-cast),
    normalized im rows moving; max-over-i from PSUM on DVE; 1/|s| folded
    in post-max on ACT; 0/1 G matmuls accumulate scoresT [128 x 32] per
    half; s norms are computed on-device from a sharded row slice and
    AllGathered; final per-core stats AllGathered (768 floats) and the
    scalar loss computed redundantly on every core.
"""

import math
import os
import sys

import numpy as np

for _p in ("/opt/trn_rl_repo", "/root/.axon_site/_ro/trn_rl_repo"):
    if os.path.isdir(_p) and _p not in sys.path:
        sys.path.append(_p)

import ml_dtypes

import concourse.bass as bass
import concourse.mybir as mybir
import concourse.tile as tile
from concourse import bacc
from concourse.bass_utils import run_bass_kernel_spmd


def _ensure_axon_hooks():
    """Some agent images ship an ``antenv`` without ``axon_hooks``, but
    bass_utils hard-imports it when trace=True.  Provide the registry and,
    when libaxon_pjrt.so is available, the real NTFF profile hook."""
    import types

    try:
        import antenv.axon_hooks  # noqa: F401
        return
    except ImportError:
        pass
    try:
        import antenv
    except ImportError:
        return
    mod = types.ModuleType("antenv.axon_hooks")
    mod._hook = None
    mod.set_axon_ntff_profile_hook = lambda h: setattr(mod, "_hook", h)
    mod.get_axon_ntff_profile_hook = lambda: mod._hook
    sys.modules["antenv.axon_hooks"] = mod
    antenv.axon_hooks = mod
    so_path = "/opt/axon/libaxon_pjrt.so"
    try:
        import trn_agent_boot.trn_boot as _tb
        if os.path.exists(so_path):
            mod._hook = _tb._ntff_profile_via_ctypes(so_path)
    except Exception:
        pass


_ensure_axon_hooks()

F32 = mybir.dt.float32
F32R = mybir.dt.float32r
BF16 = mybir.dt.bfloat16
I32 = mybir.dt.int32
AX = mybir.AxisListType
ALU = mybir.AluOpType
ACT = mybir.ActivationFunctionType
BF = ml_dtypes.bfloat16

NCORES = 8
B, LI, LS, D = 256, 36, 30, 1024
KC = D // 128               # 8 contraction chunks
MARGIN, EPS, NEG = 0.2, 1e-12, -1.0e9

SSC_T = 9                   # tile after which the sscale chain is emitted
DRAIN_LAG = 2               # G-matmul for tile t emitted at loop step t+2

LAST_RESULT = None  # BassKernelResults of the most recent run (for test harness)
DEBUG = os.environ.get("KDBG", "0") == "1"
DBG_T = int(os.environ.get("KDBG_T", "0"))   # which tile's mx to dump


# ---------------------------------------------------------------------------
# layout plan (depends only on im_len / s_len)
# ---------------------------------------------------------------------------

def make_plan(im_len, s_len):
    im_l = (np.asarray(im_len).astype(np.int64) - 1)    # 9..36 valid regions
    s_l = (np.asarray(s_len).astype(np.int64) - 3)      # 5..30 valid words
    # image slots: sort desc, deal rank-groups of 8 across cores
    order = np.argsort(-im_l, kind="stable")
    assign = order.reshape(32, NCORES)                  # [slot, core] -> b
    gmax = im_l[assign].max(axis=1)
    # quantize to mult of 4; strictly > im_l when im_l < LI (zero-pad row
    # in-slot reproduces the reference max-with-0)
    slot_len = np.where(gmax == LI, LI, np.minimum(LI, 4 * ((gmax + 4) // 4)))
    slot_off = np.concatenate([[0], np.cumsum(slot_len)])
    SL = int(slot_off[-1])
    BIc = ((SL + 127) // 128) * 128
    NRT = BIc // 128
    # chunks: greedy pack slots into <=512-col PSUM banks, split at slots
    bounds = []
    cur_start = 0
    s0 = 0
    for r in range(32):
        if slot_off[r + 1] - cur_start > 512:
            bounds.append((cur_start, s0, r))
            cur_start = int(slot_off[r])
            s0 = r
    bounds.append((cur_start, s0, 32))
    chunks = []
    for noff, cs, se in bounds:
        runs = []
        r = cs
        while r < se:
            L = int(slot_len[r])
            cnt = 1
            while r + cnt < se and slot_len[r + cnt] == L:
                cnt += 1
            runs.append((int(slot_off[r]), L, cnt, r))
            r += cnt
        chunks.append((noff, int(slot_off[se] - noff), runs))
    # sentence packing: per half, all valid (c,j) rows then pad to 128
    cj_rows = []
    half_nt = []
    for h in range(2):
        for c in range(128 * h, 128 * h + 128):
            for j in range(int(s_l[c])):
                cj_rows.append((c, 1 + j))
        while len(cj_rows) % 128:
            cj_rows.append(None)
        half_nt.append(len(cj_rows) // 128)
    NT0 = half_nt[0]
    NT = half_nt[1]
    SH = NT * 16                                        # norm-shard rows/core
    SNR_PARTS = (SH + 127) // 128
    sig = (NT0, NT, SL, BIc, tuple(int(x) for x in slot_len))
    return dict(im_l=im_l, s_l=s_l, assign=assign, slot_len=slot_len,
                slot_off=slot_off, SL=SL, BIc=BIc, NRT=NRT, chunks=chunks,
                cj_rows=cj_rows, NT0=NT0, NT=NT, SH=SH,
                SNR_PARTS=SNR_PARTS, sig=sig)


# ---------------------------------------------------------------------------
# device program
# ---------------------------------------------------------------------------

def build_nc(plan):
    NT, NT0 = plan["NT"], plan["NT0"]
    NRT, BIc, SL = plan["NRT"], plan["BIc"], plan["SL"]
    SH, SNR_PARTS = plan["SH"], plan["SNR_PARTS"]
    chunks = plan["chunks"]
    SNR_FULL, SNR_REM = SH // 128, SH % 128

    nc = bacc.Bacc(None, target_bir_lowering=False, debug=False, num_devices=NCORES)

    imr_e = nc.declare_dram_parameter("imr", [BIc, D], F32, isOutput=False)
    snr_e = nc.declare_dram_parameter("snr", [SNR_PARTS * 128, D], BF16, isOutput=False)
    st_e = nc.declare_dram_parameter("st", [NT, 128, KC, 128], BF16, isOutput=False)
    ident_e = nc.declare_dram_parameter("ident", [128, 128], F32, isOutput=False)
    gmat_e = nc.declare_dram_parameter("gmat", [128, NT * 128], F32R, isOutput=False)
    pos0_e = nc.declare_dram_parameter("pos0", [128, 32], F32, isOutput=False)
    pos1_e = nc.declare_dram_parameter("pos1", [128, 32], F32, isOutput=False)
    post0_e = nc.declare_dram_parameter("post0", [32, 128], F32, isOutput=False)
    post1_e = nc.declare_dram_parameter("post1", [32, 128], F32, isOutput=False)
    out_e = nc.declare_dram_parameter("out", [1, 1], F32, isOutput=True)
    if DEBUG:
        dbgsc_e = nc.declare_dram_parameter("dbgsc", [128, NT], F32, isOutput=True)
        dbgmx_e = nc.declare_dram_parameter("dbgmx", [128, 32], F32, isOutput=True)
        dbgS_e = nc.declare_dram_parameter("dbgS", [128, 64], F32, isOutput=True)
        dbgpay_e = nc.declare_dram_parameter("dbgpay", [128, 6], F32, isOutput=True)

    with tile.TileContext(nc) as tc:
        from contextlib import ExitStack

        with ExitStack() as ctx:
            dram = ctx.enter_context(tc.tile_pool(name="dram", bufs=1, space="DRAM"))
            const = ctx.enter_context(tc.tile_pool(name="const", bufs=1))
            small = ctx.enter_context(tc.tile_pool(name="small", bufs=1))
            stp = ctx.enter_context(tc.tile_pool(name="stp", bufs=6))
            mxp = ctx.enter_context(tc.tile_pool(name="mxp", bufs=1))
            prep = ctx.enter_context(tc.tile_pool(name="prep", bufs=3))
            # PSUM budget (8 banks): align 5 + S accumulators 2 + epi scratch 1
            pal = ctx.enter_context(tc.tile_pool(name="pal", bufs=5, space="PSUM"))

            # DRAM scratch
            snorm_d = dram.tile([SH, 1], F32, tag="snorm_d")
            snormall_d = dram.tile([NT * 128, 1], F32, tag="snormall_d")
            pay_d = dram.tile([128, 6], F32, tag="pay_d")
            ag2_d = dram.tile([NCORES * 128, 6], F32, tag="ag2_d")

            def epi_psum(shape, name):
                return pal.tile(shape, F32, tag="epi", bufs=1, name=name)

            ident = const.tile([128, 128], F32, tag="ident")
            nc.sync.dma_start(out=ident[:, :], in_=ident_e[:, :])

            # ---- phase 1: im loads + squares (ACT), s norm shard (GpSimd/DVE)
            imr_tiles = []
            imssqs = []
            for rt in range(NRT):
                imr_t = prep.tile([128, D], F32, tag="imld", name="imr_t", bufs=NRT)
                nc.sync.dma_start(out=imr_t[:, :], in_=imr_e[128 * rt:128 * (rt + 1), :])
                sq = prep.tile([128, D], F32, tag="imsq", name="sq")
                ssq = small.tile([128, 1], F32, tag=f"imssq{rt}", name="ssq")
                nc.scalar.activation(sq[:, :], imr_t[:, :], ACT.Square,
                                     accum_out=ssq[:, :])
                imr_tiles.append(imr_t)
                imssqs.append(ssq)

            # sharded s sum-of-squares + AllGather
            snr_t = const.tile([128, SNR_PARTS * D], BF16, tag="snr_t")
            nc.sync.dma_start(
                out=snr_t.rearrange("p (j d) -> p j d", j=SNR_PARTS),
                in_=snr_e.ap().rearrange("(j p) d -> p j d", p=128),
            )
            snr_sq = const.tile([128, SNR_PARTS * D], F32, tag="snr_sq")
            nc.gpsimd.tensor_mul(snr_sq[:, :], snr_t[:, :], snr_t[:, :])
            ssqP = small.tile([128, SNR_PARTS], F32, tag="ssqP")
            nc.vector.tensor_reduce(
                out=ssqP[:, :],
                in_=snr_sq.rearrange("p (j d) -> p j d", j=SNR_PARTS),
                axis=AX.X, op=ALU.add,
            )
            ssqT_ps = epi_psum([SNR_PARTS, 128], "ssqT_ps")
            nc.tensor.transpose(ssqT_ps[:, :], ssqP[:, :], ident[:, :])
            ssqT = small.tile([SNR_PARTS, 128], F32, tag="ssqT")
            nc.scalar.copy(ssqT[:, :], ssqT_ps[:, :])
            nc.sync.dma_start(
                out=snorm_d[0:SNR_FULL * 128, :]
                .rearrange("(j p) o -> j (p o)", j=SNR_FULL),
                in_=ssqT[0:SNR_FULL, :],
            )
            if SNR_REM:
                nc.sync.dma_start(
                    out=snorm_d[SNR_FULL * 128:SH, :]
                    .rearrange("(j p) o -> j (p o)", j=1),
                    in_=ssqT[SNR_FULL:SNR_FULL + 1, 0:SNR_REM],
                )
            nc.gpsimd.collective_compute(
                "AllGather", ALU.bypass,
                replica_groups=[list(range(NCORES))],
                ins=[snorm_d.opt()],
                outs=[snormall_d.opt()],
            )

            # bf16 identity for fast prep transposes
            ident_bf = const.tile([128, 128], BF16, tag="ident_bf")
            nc.scalar.copy(ident_bf[:, :], ident[:, :])

            # finish im prep: scale (DVE), cast to bf16, transpose (PE, bf16)
            imt = const.tile([128, KC * BIc], BF16, tag="imt")
            for rt in range(NRT):
                imr_t = imr_tiles[rt]
                ssq = imssqs[rt]
                nrm = small.tile([128, 1], F32, tag="imnrm")
                nc.scalar.activation(nrm[:, :], ssq[:, :], ACT.Sqrt)
                nc.vector.tensor_scalar_max(nrm[:, :], nrm[:, :], EPS)
                rcp = small.tile([128, 1], F32, tag="imrcp")
                nc.vector.reciprocal(rcp[:, :], nrm[:, :])
                ims_bf = prep.tile([128, D], BF16, tag="imsbf", name="ims_bf")
                nc.vector.tensor_scalar(
                    out=ims_bf[:, :], in0=imr_t[:, :], scalar1=rcp[:, :],
                    scalar2=None, op0=ALU.mult,
                )
                for k in range(KC):
                    pst = pal.tile([128, 128], BF16, tag="al", name="pst")
                    nc.tensor.transpose(pst[:, :], ims_bf[:, 128 * k:128 * (k + 1)],
                                        ident_bf[:, :])
                    dst = imt[:, BIc * k + 128 * rt:BIc * k + 128 * (rt + 1)]
                    if k % 2 == 0:
                        nc.vector.tensor_copy(dst, pst[:, :])
                    else:
                        nc.scalar.copy(dst, pst[:, :])

            # ---- late consts (needed by G-matmuls / epilogue) ----
            gmat = const.tile([128, NT * 128], F32R, tag="gmat")
            nc.sync.dma_start(out=gmat[:, :], in_=gmat_e[:, :])
            pos0 = const.tile([128, 32], F32, tag="pos0")
            nc.sync.dma_start(out=pos0[:, :], in_=pos0_e[:, :])
            pos1 = const.tile([128, 32], F32, tag="pos1")
            nc.sync.dma_start(out=pos1[:, :], in_=pos1_e[:, :])
            post0 = const.tile([32, 128], F32, tag="post0")
            nc.sync.dma_start(out=post0[:, :], in_=post0_e[:, :])
            post1 = const.tile([32, 128], F32, tag="post1")
            nc.sync.dma_start(out=post1[:, :], in_=post1_e[:, :])
            ones128 = const.tile([128, 1], F32, tag="ones128")
            nc.gpsimd.memset(ones128[:, :], 1.0)
            margin128 = const.tile([128, 1], F32, tag="margin128")
            nc.gpsimd.memset(margin128[:, :], MARGIN)

            # ---- main loop over NT packed M-tiles ----
            # S halves share one PSUM bank: cols [0:32] half0, [32:64] half1
            psacc = ctx.enter_context(tc.tile_pool(name="psacc", bufs=1, space="PSUM"))
            s_ps = [psacc.tile([128, 32], F32, tag=f"S{h}", name=f"S{h}")[:, :]
                    for h in range(2)]

            posm = [pos0, pos1]
            payload = small.tile([128, 6], F32, tag="payload")
            snd = [small.tile([128, 32], F32, tag=f"snd{h}", name=f"snd{h}")
                   for h in range(2)]
            trash = small.tile([128, 32], F32, tag="trash")
            negm = [small.tile([128, 32], F32, tag=f"negm{h}", name=f"negm{h}")
                    for h in range(2)]
            nc.vector.tensor_scalar_mul(negm[0][:, :], pos0[:, :], NEG)
            nc.vector.tensor_scalar_mul(negm[1][:, :], pos1[:, :], NEG)
            posr = [small.tile([128, 32], F32R, tag=f"posr{h}", name=f"posr{h}")
                    for h in range(2)]
            nc.scalar.copy(posr[0][:, :], pos0[:, :])
            nc.scalar.copy(posr[1][:, :], pos1[:, :])
            postr = [small.tile([32, 128], F32R, tag=f"postr{h}", name=f"postr{h}")
                     for h in range(2)]
            nc.scalar.copy(postr[0][:, :], post0[:, :])
            nc.scalar.copy(postr[1][:, :], post1[:, :])
            onesr = const.tile([128, 1], F32R, tag="onesr")
            nc.scalar.copy(onesr[:, :], ones128[:, :])
            rm = small.tile([32, 2], F32, tag="rm")

            def emit_stats_h(h):
                # diag extraction: accum_out = sum(S * pos) -> payload col 2+h
                nc.vector.scalar_tensor_tensor(
                    out=trash[:, :], in0=s_ps[h], scalar=1.0, in1=posm[h][:, :],
                    op0=ALU.mult, op1=ALU.mult, accum_out=payload[:, 2 + h:3 + h],
                )
                nc.vector.tensor_add(snd[h][:, :], s_ps[h], negm[h][:, :])
                nc.vector.tensor_reduce(out=payload[:, h:h + 1], in_=snd[h][:, :],
                                        axis=AX.X, op=ALU.max)
                stp_ps = epi_psum([32, 128], "stp_ps")
                nc.tensor.transpose(stp_ps[:, :], snd[h][:, :], ident[:, :])
                nc.vector.tensor_reduce(out=rm[:, h:h + 1], in_=stp_ps[:, :],
                                        axis=AX.X, op=ALU.max)

            def emit_scale_g(t):
                mx = pending[t]
                h = 0 if t < NT0 else 1
                mx_r = mxp.tile([128, 32], F32R, tag="mx_r", name="mx_r", bufs=4)
                nc.scalar.mul(mx_r[:, :], mx[:, :], mul=sscale[:, t:t + 1])
                nc.tensor.matmul(
                    s_ps[h],
                    lhsT=gmat[:, 128 * t:128 * (t + 1)],
                    rhs=mx_r[:, :],
                    start=(t == 0 or t == NT0),
                    stop=(t == NT0 - 1 or t == NT - 1),
                )

            pending = {}
            next_g = [0]

            def drain_g(upto):
                while next_g[0] <= upto:
                    emit_scale_g(next_g[0])
                    next_g[0] += 1

            for t in range(NT):
                st_t = stp.tile([128, KC * 128], BF16, tag="st")
                nc.sync.dma_start(
                    out=st_t.rearrange("p (k c) -> p k c", k=KC),
                    in_=st_e[t, :, :, :],
                )
                psc = [pal.tile([128, nw], F32, tag="al", name="ps")
                       for (_, nw, _) in chunks]
                for k in range(KC):
                    for ci, (noff, nw, _) in enumerate(chunks):
                        nc.tensor.matmul(
                            psc[ci][:, :],
                            lhsT=st_t[:, 128 * k:128 * (k + 1)],
                            rhs=imt[:, BIc * k + noff:BIc * k + noff + nw],
                            start=(k == 0), stop=(k == KC - 1),
                        )
                mx = mxp.tile([128, 32], F32, tag="mx", name="mx", bufs=NT)
                for ci, (noff, nw, runs) in enumerate(chunks):
                    for (off, L, cnt, slot0) in runs:
                        nc.vector.tensor_reduce(
                            out=mx[:, slot0:slot0 + cnt],
                            in_=psc[ci][:, off - noff:off - noff + cnt * L]
                            .rearrange("p (g i) -> p g i", i=L),
                            axis=AX.X, op=ALU.max,
                        )
                pending[t] = mx
                if DEBUG and t == DBG_T:
                    nc.sync.dma_start(out=dbgmx_e[:, :], in_=mx[:, :])

                if t == SSC_T:
                    # ---- s scale vector [128, NT] from the AllGather ----
                    snall = small.tile([NT, 128], F32, tag="snall")
                    nc.sync.dma_start(
                        out=snall[:, :],
                        in_=snormall_d.rearrange("(t p) o -> t (p o)", t=NT),
                    )
                    ssqall_ps = epi_psum([128, NT], "ssqall_ps")
                    nc.tensor.transpose(ssqall_ps[:, :], snall[:, :],
                                        ident[0:NT, 0:NT])
                    ssqall = small.tile([128, NT], F32, tag="ssqall")
                    nc.scalar.copy(ssqall[:, :], ssqall_ps[:, :])
                    snrm = small.tile([128, NT], F32, tag="snrm")
                    nc.scalar.activation(snrm[:, :], ssqall[:, :], ACT.Sqrt)
                    nc.vector.tensor_scalar_max(snrm[:, :], snrm[:, :], EPS)
                    sscale = small.tile([128, NT], F32, tag="sscale")
                    nc.vector.reciprocal(sscale[:, :], snrm[:, :])
                    if DEBUG:
                        nc.sync.dma_start(out=dbgsc_e[:, :], in_=sscale[:, :])

                if t > SSC_T:
                    drain_g(t - DRAIN_LAG)
                if t == NT0 + 3:
                    emit_stats_h(0)
            drain_g(NT - 1)

            # ---- epilogue: half-1 stats + AllGather + final ----
            emit_stats_h(1)
            if DEBUG:
                sdump = small.tile([128, 64], F32, tag="sdump")
                nc.vector.tensor_copy(sdump[:, 0:32], s_ps[0])
                nc.vector.tensor_copy(sdump[:, 32:64], s_ps[1])
                nc.sync.dma_start(out=dbgS_e[:, :], in_=sdump[:, :])
            # diag per slot: pair pay col 2 with pos0 and col 3 with pos1 in
            # SEPARATE rhs columns (a core owns images of both halves, so a
            # shared 2-col rhs would contaminate: diag0[g] + diag1[g])
            dcol4 = small.tile([128, 4], F32R, tag="dcol4")
            nc.scalar.copy(dcol4[:, 0:1], payload[:, 2:3])
            nc.scalar.mul(dcol4[:, 1:2], payload[:, 2:3], mul=0.0)
            nc.scalar.copy(dcol4[:, 2:3], payload[:, 3:4])
            nc.scalar.mul(dcol4[:, 3:4], payload[:, 3:4], mul=0.0)
            rowmax = small.tile([32, 1], F32, tag="rowmax")
            nc.vector.tensor_max(rowmax[:, :], rm[:, 0:1], rm[:, 1:2])
            dfree_ps = epi_psum([32, 2], "dfree_ps")
            nc.tensor.matmul(dfree_ps[:, :], lhsT=posr[0][:, :],
                             rhs=dcol4[:, 0:2], start=True, stop=False)
            nc.tensor.matmul(dfree_ps[:, :], lhsT=posr[1][:, :],
                             rhs=dcol4[:, 2:4], start=False, stop=True)
            dfree_sb = small.tile([32, 1], F32, tag="dfree_sb")
            nc.scalar.copy(dfree_sb[:, :], dfree_ps[:, 0:1])
            rh_pre = small.tile([32, 2], F32, tag="rh_pre")
            nc.gpsimd.memset(rh_pre[:, :], 0.0)
            nc.vector.tensor_sub(rh_pre[:, 0:1], rowmax[:, :], dfree_sb[:, :])
            rowhinge = small.tile([32, 2], F32R, tag="rowhinge")
            nc.scalar.activation(rowhinge[:, :], rh_pre[:, :], ACT.Relu,
                                 bias=margin128[0:32, :])
            for h in range(2):
                rh_ps = epi_psum([128, 2], "rh_ps")
                nc.tensor.matmul(rh_ps[:, :], lhsT=postr[h][:, :],
                                 rhs=rowhinge[:, :], start=True, stop=True)
                nc.scalar.copy(payload[:, 4 + h:5 + h], rh_ps[:, 0:1])

            # payload -> DRAM (one DMA, contiguous per partition) -> AllGather
            if DEBUG:
                nc.sync.dma_start(out=dbgpay_e[:, :], in_=payload[:, :])
            nc.sync.dma_start(out=pay_d[:, :], in_=payload[:, :])
            nc.gpsimd.collective_compute(
                "AllGather", ALU.bypass,
                replica_groups=[list(range(NCORES))],
                ins=[pay_d.opt()],
                outs=[ag2_d.opt()],
            )

            # final combine (identical on every core): strided reload puts
            # sentences on partitions, (stat, core) along free
            agg_in = small.tile([128, 6 * NCORES], F32, tag="agg_in")
            nc.sync.dma_start(
                out=agg_in.rearrange("p (c m) -> p c m", m=NCORES),
                in_=ag2_d.rearrange("(m p) c -> p c m", m=NCORES),
            )
            agv = agg_in.rearrange("p (c m) -> p c m", m=NCORES)
            agg = small.tile([128, 6], F32, tag="agg")
            nc.vector.tensor_reduce(out=agg[:, 0:2], in_=agv[:, 0:2, :],
                                    axis=AX.X, op=ALU.max)
            nc.vector.tensor_reduce(out=agg[:, 2:6], in_=agv[:, 2:6, :],
                                    axis=AX.X, op=ALU.add)
            finalvec = small.tile([128, 4], F32R, tag="finalvec")
            ch2 = small.tile([128, 2], F32, tag="ch2")
            nc.vector.tensor_sub(ch2[:, :], agg[:, 0:2], agg[:, 2:4])
            nc.scalar.activation(finalvec[:, 0:2], ch2[:, :], ACT.Relu,
                                 bias=margin128[:, :])
            nc.scalar.copy(finalvec[:, 2:4], agg[:, 4:6])
            fin_ps = epi_psum([1, 4], "fin_ps")
            nc.tensor.matmul(fin_ps[:, :], lhsT=onesr[:, :],
                             rhs=finalvec[:, :], start=True, stop=True)
            loss = small.tile([1, 1], F32, tag="loss")
            nc.vector.tensor_reduce(out=loss[:, :], in_=fin_ps[:, :], axis=AX.X,
                                    op=ALU.add)
            nc.sync.dma_start(out=out_e[:, :], in_=loss[:, :])

    nc.finalize()
    return nc


# ---------------------------------------------------------------------------
# host side
# ---------------------------------------------------------------------------

def build_in_maps(plan, im_set, s_seq):
    im_set = np.asarray(im_set, dtype=np.float32)
    s_seq = np.asarray(s_seq, dtype=np.float32)
    NT, NT0, BIc = plan["NT"], plan["NT0"], plan["BIc"]
    SH, SNR_PARTS = plan["SH"], plan["SNR_PARTS"]
    cj = plan["cj_rows"]
    CJc = NT * 128

    s_rows = np.zeros((CJc, D), np.float32)
    idx = [i for i, row in enumerate(cj) if row is not None]
    cs = np.array([cj[i][0] for i in idx])
    js = np.array([cj[i][1] for i in idx])
    s_rows[idx] = s_seq[cs, js]
    st = np.ascontiguousarray(
        s_rows.reshape(NT, 128, KC, 128).transpose(0, 3, 2, 1)).astype(BF)
    s_rows_bf = s_rows.astype(BF)

    gmat = np.zeros((128, NT * 128), np.float32)
    for t in range(NT):
        h = 0 if t < NT0 else 1
        for p in range(128):
            row = cj[128 * t + p]
            if row is not None:
                gmat[p, 128 * t + (row[0] - 128 * h)] = 1.0
    ident = np.eye(128, dtype=np.float32)

    in_maps = []
    for m in range(NCORES):
        imr = np.zeros((BIc, D), np.float32)
        pos0 = np.zeros((128, 32), np.float32)
        pos1 = np.zeros((128, 32), np.float32)
        for r in range(32):
            g = int(plan["assign"][r, m])
            l = int(plan["im_l"][g])
            o = int(plan["slot_off"][r])
            imr[o:o + l] = im_set[g, 1:1 + l]
            (pos0 if g < 128 else pos1)[g % 128, r] = 1.0
        snr = np.zeros((SNR_PARTS * 128, D), BF)
        snr[0:SH] = s_rows_bf[SH * m:SH * (m + 1)]
        in_maps.append({
            "imr": imr,
            "snr": snr,
            "st": st,
            "ident": ident,
            "gmat": gmat,
            "pos0": pos0,
            "pos1": pos1,
            "post0": np.ascontiguousarray(pos0.T),
            "post1": np.ascontiguousarray(pos1.T),
        })
    return in_maps


_NC_CACHE = {}


def kernel(im_set, s_seq, im_len, s_len):
    global LAST_RESULT
    plan = make_plan(im_len, s_len)
    nc = _NC_CACHE.get(plan["sig"])
    if nc is None:
        nc = build_nc(plan)
        _NC_CACHE[plan["sig"]] = nc
    in_maps = build_in_maps(plan, im_set, s_seq)
    res = run_bass_kernel_spmd(nc, in_maps, core_ids=list(range(NCORES)))
    LAST_RESULT = res
    out = np.asarray(res.results[0]["out"], dtype=np.float32).reshape(())
    return out
